# revision 60
# baseline (speedup 1.0000x reference)
"""Trainium2 Bass kernel for nn_MoEGate_6150393168540 (moe_routing).

Computes, for x [B=65536, D=1024], gate/expert weights [E=8, D] and biases [E]:
    gate = softmax(x @ gate_w.T + gate_b)            # [B, 8]
    keep top-k (k=2) gate values, zero the rest (no renormalization)
    expert = x @ expert_w.T + expert_b               # [B, 8]
    out = sum(gate_masked * expert, axis=1)          # [B, 1]

Strategy (8 NeuronCores, data-parallel over the batch):
  - Each core gets 8192 rows of x; weights are replicated.
  - The D-contraction needs x transposed (D on partitions). PE transposes x in
    fp32 ([128,128] blocks -> PSUM, bit-exact).
  - Scores must order-match a pure-fp32 reference (top-2 selection), so the
    matmul uses an exact fp16 Dekker split: hi = bf16(xT) (a u16 bit-slice
    copy on ACT), lo = fp16(xT - hi) (DVE). Weights are split host-side the
    same way. hi*w_hi, hi*w_lo, lo*w_hi are exact products accumulated in
    fp32 PSUM => scores accurate to ~1e-7, at fp16 matmul speed.
  - mm_hi: lhsT=[w_hi|w_lo] [128,32]; mm_lo accumulates into the same rows.
  - Scores come back to batch-major WITHOUT the PE: z is copied to SBUF as an
    exact fp16 (hi, lo) pair, the DMA xbar engine transposes the 2-byte
    planes (dma_start_transpose), and two DVE adds reconstruct the exact
    fp32 score sum into an SBUF group buffer. This keeps the PE stream free
    for transposes + Dekker matmuls only.
  - Postprocess per group [128 rows, 32 g, 16]: +bias, exp on ACT, top-2
    threshold via a min/max tournament tree, mask, weighted sum, divide by
    the softmax denominator; DVE 32x32 block transpose makes the output DMA
    contiguous.
"""

import sys

sys.path.insert(0, "/opt/trn_rl_repo")

from contextlib import ExitStack

import numpy as np

import concourse.bass as bass
import concourse.mybir as mybir
import concourse.tile as tile

F32 = mybir.dt.float32
F16 = mybir.dt.float16
BF16 = mybir.dt.bfloat16
U16 = mybir.dt.uint16
ALU = mybir.AluOpType
AXX = mybir.AxisListType.X
EXP = mybir.ActivationFunctionType.Exp

B, D, E = 65536, 1024, 8
N_CORES = 8
B_LOC = B // N_CORES  # 8192
BLK = 512  # rows per block
DC = D // 128  # 8 contraction chunks


def split_waits(nc, max_waits: int = 1) -> int:
    """walrus here allows only one semaphore wait per instruction; hoist the
    rest into preceding single-wait NOPs on the same engine (engine streams
    execute in order, so earlier waits on the same engine are equivalent)."""
    n_split = 0
    for f in nc.m.functions:
        for bb in f.blocks:
            new = []
            for inst in bb.instructions:
                si = inst.sync_info
                if si is not None and si.on_wait and len(si.on_wait) > max_waits:
                    waits = list(si.on_wait)
                    for w in waits[:-max_waits]:
                        n_split += 1
                        nop = mybir.InstNoOp(name=f"{inst.name}-ws{n_split}")
                        nop.engine = inst.engine
                        nop.sync_info = mybir.SyncInfo(on_wait=[w], on_update=[])
                        new.append(nop)
                    inst.sync_info = mybir.SyncInfo(
                        on_wait=waits[-max_waits:], on_update=list(si.on_update or [])
                    )
                new.append(inst)
            bb.instructions = new
    return n_split


def build_module(b_loc: int = B_LOC, split: bool = True):
    assert b_loc % 4096 == 0
    nc = bass.Bass()
    x = nc.dram_tensor("x", [b_loc, D], F32, kind="ExternalInput")
    whl = nc.dram_tensor("whl", [128, DC, 32], F16, kind="ExternalInput")
    brow = nc.dram_tensor("brow", [512], F32, kind="ExternalInput")
    fold = nc.dram_tensor("fold", [32, 16], F16, kind="ExternalInput")
    ident = nc.dram_tensor("ident", [128, 128], F32, kind="ExternalInput")
    y = nc.dram_tensor("y", [b_loc], F32, kind="ExternalOutput")

    n_blk = b_loc // BLK
    tt = nc.vector.tensor_tensor

    with tile.TileContext(nc) as tc, ExitStack() as ctx:
        consts = ctx.enter_context(tc.tile_pool(name="consts", bufs=1))
        xpool = ctx.enter_context(tc.tile_pool(name="xpool", bufs=4))
        xh_pool = ctx.enter_context(tc.tile_pool(name="xh", bufs=12))
        xl_pool = ctx.enter_context(tc.tile_pool(name="xl", bufs=12))
        z32_pool = ctx.enter_context(tc.tile_pool(name="z32", bufs=3))
        ztr_pool = ctx.enter_context(tc.tile_pool(name="ztr", bufs=3))
        zt_pool = ctx.enter_context(tc.tile_pool(name="ztsb", bufs=2))
        pp = ctx.enter_context(tc.tile_pool(name="pp", bufs=3))
        xt_pool = ctx.enter_context(tc.tile_pool(name="xtps", bufs=4, space="PSUM"))
        z_pool = ctx.enter_context(tc.tile_pool(name="zps", bufs=3, space="PSUM"))
        warm_pool = ctx.enter_context(tc.tile_pool(name="warm", bufs=1, space="PSUM"))

        ident_sb = consts.tile([128, 128], F32)
        whl_sb = consts.tile([128, DC, 32], F16)
        bias_sb = consts.tile([128, 512], F32)
        fold_sb = consts.tile([32, 16], F16)
        warm_sb = consts.tile([128, 1], BF16)

        def postprocess(zt_h, G, g0, outv, eng=None):
            # zt_h [128, G, 16] = [128 rows, G row-groups, 8 gate | 8 expert];
            # one slice of a 4096-row window, writing outv[:, g0:g0+G]
            eng = eng or nc.vector
            et = eng.tensor_tensor
            sfx = str(G) + ("g" if eng is nc.gpsimd else "")
            zb = pp.tile([128, G, 16], F32, name="zb", tag="zb" + sfx)
            eng.tensor_add(
                zb,
                zt_h,
                bias_sb.rearrange("p (g e) -> p g e", e=16)[:, 0:G, :],
            )
            g8 = zb[:, :, 0:8]
            y8 = zb[:, :, 8:16]
            p8 = pp.tile([128, G, 8], F32, name="p8", tag="p8" + sfx)
            nc.scalar.activation(p8, g8, EXP)
            den = pp.tile([128, G], F32, name="den", tag="den" + sfx)
            nc.vector.tensor_reduce(den, p8, axis=AXX, op=ALU.add)
            # top-2 threshold: tournament keeping (max, 2nd max) per segment
            h1 = pp.tile([128, G, 4], F32, name="h1", tag="h1" + sfx)
            l1 = pp.tile([128, G, 4], F32, name="l1", tag="l1" + sfx)
            et(h1, p8[:, :, 0:4], p8[:, :, 4:8], op=ALU.max)
            et(l1, p8[:, :, 0:4], p8[:, :, 4:8], op=ALU.min)
            h2 = pp.tile([128, G, 2], F32, name="h2", tag="h2" + sfx)
            v2 = pp.tile([128, G, 2], F32, name="v2", tag="v2" + sfx)
            u2 = pp.tile([128, G, 2], F32, name="u2", tag="u2" + sfx)
            m2q = pp.tile([128, G, 2], F32, name="m2q", tag="m2q" + sfx)
            et(h2, h1[:, :, 0:2], h1[:, :, 2:4], op=ALU.max)
            et(v2, h1[:, :, 0:2], h1[:, :, 2:4], op=ALU.min)
            et(u2, l1[:, :, 0:2], l1[:, :, 2:4], op=ALU.max)
            et(m2q, u2, v2, op=ALU.max)
            v3 = pp.tile([128, G, 1], F32, name="v3", tag="v3" + sfx)
            u3 = pp.tile([128, G, 1], F32, name="u3", tag="u3" + sfx)
            m2f = pp.tile([128, G, 1], F32, name="m2f", tag="m2f" + sfx)
            et(v3, h2[:, :, 0:1], h2[:, :, 1:2], op=ALU.min)
            et(u3, m2q[:, :, 0:1], m2q[:, :, 1:2], op=ALU.max)
            et(m2f, u3, v3, op=ALU.max)
            # mask & weighted sum
            msk = pp.tile([128, G, 8], F32, name="msk", tag="msk" + sfx)
            et(msk, p8, m2f.to_broadcast([128, G, 8]), op=ALU.is_ge)
            pm = pp.tile([128, G, 8], F32, name="pm", tag="pm" + sfx)
            et(pm, msk, p8, op=ALU.mult)
            prod = pp.tile([128, G, 8], F32, name="prod", tag="prod" + sfx)
            et(prod, pm, y8, op=ALU.mult)
            num = pp.tile([128, G], F32, name="num", tag="num" + sfx)
            nc.vector.tensor_reduce(num, prod, axis=AXX, op=ALU.add)
            rden = pp.tile([128, G], F32, name="rden", tag="rden" + sfx)
            nc.vector.reciprocal(rden, den)
            tt(outv[:, g0 : g0 + G], num, rden, op=ALU.mult)

        def finalize(b0, outv):
            # 32x32 block transpose so each partition holds a contiguous run
            tv = pp.tile([128, 32], F32, name="tv", tag="tv")
            nc.vector.transpose(tv, outv)
            yf = y.ap()
            for a in range(4):
                # dest[i, k] = y[b0 + 32a + 128 i + k], i,k in 0..32
                dest = bass.AP(yf.tensor, b0 + 32 * a, [[128, 32], [1, 32]])
                q = nc.sync if a % 2 == 0 else nc.scalar
                q.dma_start(out=dest, in_=tv[32 * a : 32 * a + 32, :])

        state = {"zz_g": None, "outv": None}

        def chain(zz_g, bank0, nb, outv, eng=None):
            # xbar-transpose nb blocks' exact fp16 (hi, lo) score pairs back
            # to batch-major: logical row f = 1024 b + 512 h + 128 j + p
            #   -> out[p, 8b+4h+j, :], then two exact adds + postprocess
            sfx = str(nb)
            ztr = ztr_pool.tile([128, 8 * nb, 32], F16, name="ztr", tag="ztr" + sfx)
            nc.sync.dma_start_transpose(
                out=ztr,
                in_=zz_g[:, bank0 : bank0 + nb, :, :].rearrange(
                    "q b h r -> q (b h r)"
                ),
            )
            # exact score: (hi_whi + hi_wlo) + (lo_whi + lo_wlo); fp16 pairs
            # add exactly in fp32
            t1 = ztr_pool.tile([128, 8 * nb, 16], F32, name="t1", tag="t1" + sfx)
            tt(t1, ztr[:, :, 0:16], ztr[:, :, 16:32], op=ALU.add)
            zt_h = zt_pool.tile([128, 4 * nb, 16], F32, name="zt_h", tag="zt" + sfx)
            t1v = t1.rearrange("p (b h j) e -> p b h j e", b=nb, h=2)
            tt(
                zt_h.rearrange("p (b j) e -> p b j e", b=nb),
                t1v[:, :, 0, :, :],
                t1v[:, :, 1, :, :],
                op=ALU.add,
            )
            postprocess(zt_h, 4 * nb, 4 * bank0, outv, eng=eng)

        def chain_pe(zz_g, bank, outv, eng=None):
            # PE-fold for the last blocks: the PE is idle at the tail, and
            # folding there skips the xbar DMA round-trip. One matmul pair per
            # 128-row group: out[p, e] = sum_q zz[q, 128j+p] fold[q, e]
            # = (w_hi part + w_lo part), accumulated over both fp16 transport
            # halves => exact fp32 score, batch-major in PSUM.
            xt_ps = xt_pool.tile([128, 512], F32)
            for j in range(4):
                nc.tensor.matmul(
                    xt_ps[:, 16 * j : 16 * j + 16],
                    zz_g[:, bank, 0, 128 * j : 128 * j + 128],
                    fold_sb,
                    start=True,
                    stop=False,
                )
                nc.tensor.matmul(
                    xt_ps[:, 16 * j : 16 * j + 16],
                    zz_g[:, bank, 1, 128 * j : 128 * j + 128],
                    fold_sb,
                    start=False,
                    stop=True,
                )
            postprocess(
                xt_ps[:, 0:64].rearrange("p (g e) -> p g e", e=16),
                4,
                4 * bank,
                outv,
                eng=eng,
            )

        def emit_hi(blk, c, z_ps, xh, xl):
            # hi stream (bf16 view) against the [w_hi|w_lo] fp16 pair
            nc.tensor.matmul(
                z_ps, whl_sb[:, c, :], xh.bitcast(BF16),
                start=(c == 0), stop=False,
            )

        def emit_lo(blk, c, z_ps, xh, xl):
            # lo stream (fp16): together with the hi stream this accumulates
            # the complete product (hi+lo)*(w_hi+w_lo) into rows 0:32.
            nc.tensor.matmul(
                z_ps, whl_sb[:, c, :], xl, start=False, stop=(c == DC - 1)
            )
            if c == DC - 1:
                bank_i = blk % 8
                if bank_i == 0:
                    state["zz_g"] = z32_pool.tile(
                        [32, 8, 2, 512], F16, name="zz_g", tag="zz_g"
                    )
                    state["outv"] = pp.tile(
                        [128, 32], F32, name="outv", tag="outv"
                    )
                zz_g = state["zz_g"]
                nc.scalar.copy(zz_g[:, bank_i, 0, :], z_ps)
                nc.vector.tensor_sub(
                    zz_g[:, bank_i, 1, :], z_ps, zz_g[:, bank_i, 0, :]
                )
                b0 = (blk // 8) * 4096
                outv = state["outv"]
                last_group = (blk // 8) == (n_blk // 8) - 1
                if bank_i == 3:
                    chain(zz_g, 0, 4, outv)
                elif last_group and bank_i == 5:
                    # final group: smaller tail chains so less is exposed
                    # after the last PE work
                    chain(zz_g, 4, 2, outv)
                elif last_group and bank_i == 6:
                    chain_pe(zz_g, 6, outv)
                elif bank_i == 7:
                    if last_group:
                        chain_pe(zz_g, 7, outv)
                    else:
                        chain(zz_g, 4, 4, outv)
                    finalize(b0, outv)

        pending = []
        for blk in range(n_blk):
            r0 = blk * BLK
            x_blk = xpool.tile([128, 4, D], F32, name="x_blk", tag="x_blk")
            xin = x.ap()[r0 : r0 + BLK, :].rearrange("(j p) d -> p j d", p=128)
            if blk == 0:
                # chunk-0 slice of x first on the sync queue (each DMA issue
                # costs ~650ns, so keep the count low); ident + bias go on the
                # gpsimd queue in parallel
                nc.gpsimd.memset(warm_sb, 0)
                nc.gpsimd.dma_start(out=ident_sb, in_=ident.ap())
                nc.gpsimd.dma_start(out=fold_sb, in_=fold.ap())
                nc.gpsimd.dma_start(
                    out=bias_sb,
                    in_=brow.ap().unsqueeze(0).to_broadcast([128, 512]),
                )
                nc.sync.dma_start(out=x_blk[:, :, 0:128], in_=xin[:, :, 0:128])
                nc.sync.dma_start(out=whl_sb, in_=whl.ap())
                nc.sync.dma_start(
                    out=x_blk[:, :, 128:256], in_=xin[:, :, 128:256]
                )
                nc.sync.dma_start(
                    out=x_blk[:, :, 256:512], in_=xin[:, :, 256:512]
                )
                nc.sync.dma_start(
                    out=x_blk[:, :, 512:1024], in_=xin[:, :, 512:1024]
                )
                # warm-up matmuls: transpose-mode doesn't count as PE activity
                # for the HAM clock gate, so run ~3.4us of broadcast-operand
                # matmuls during the DMA wait to reach 2.4 GHz before the real
                # stream starts
                warm_ps = warm_pool.tile([2, 512], F32)
                for _ in range(8):
                    nc.tensor.matmul(
                        warm_ps,
                        warm_sb.to_broadcast([128, 2]),
                        warm_sb.to_broadcast([128, 512]),
                        start=True,
                        stop=True,
                    )
            else:
                nc.sync.dma_start(out=x_blk, in_=xin)
            z_ps = z_pool.tile([32, 512], F32)
            for cc in range(0, DC, 4):
                # group 4 chunks: PE runs [16 transposes][8 Dekker MMs] to cut
                # the fp32-transpose-mode <-> 16-bit-matmul switches and the
                # chunk-boundary LDW exposure 4x
                splits = []
                for c in range(cc, cc + 4):
                    xt_ps = xt_pool.tile([128, 512], F32)
                    for j in range(4):
                        nc.tensor.transpose(
                            xt_ps[:, 128 * j : 128 * j + 128],
                            x_blk[:, j, 128 * c : 128 * c + 128],
                            ident_sb,
                        )
                    splits.append((c, xt_ps))
                if blk == 0 and cc == 0:
                    # keep the PE (and the HAM activity window) busy while the
                    # rest of block 0 is still in flight on the DMA
                    warm_ps = warm_pool.tile([2, 512], F32)
                    for _ in range(10):
                        nc.tensor.matmul(
                            warm_ps,
                            warm_sb.to_broadcast([128, 2]),
                            warm_sb.to_broadcast([128, 512]),
                            start=True,
                            stop=True,
                        )
                for c, xt_ps in splits:
                    # hi = bf16 truncation of xT: a pure u16 bit-slice copy on
                    # ACT (psum->sbuf); lo = fp16(xT - hi) on DVE.
                    xt_hi_view = (
                        xt_ps.bitcast(U16)
                        .rearrange("p (k two) -> p k two", two=2)[:, :, 1]
                    )
                    xh = xh_pool.tile([128, 512], U16)
                    nc.scalar.copy(xh, xt_hi_view)
                    xl = xl_pool.tile([128, 512], F16)
                    nc.vector.tensor_sub(xl, xt_ps, xh.bitcast(BF16))
                    pending.append((blk, c, z_ps, xh, xl))
                keep = 2 if blk >= n_blk - 2 else 6
                while len(pending) > keep:
                    emit_hi(*pending[0])
                    emit_lo(*pending.pop(0))
        for args in pending:
            emit_hi(*args)
            emit_lo(*args)

    if split:
        split_waits(nc)
    return nc


def host_inputs(gate_w, gate_b, expert_w, expert_b):
    """Host-side prep of the small replicated tensors."""
    W = np.concatenate([gate_w, expert_w], axis=0).astype(np.float32)  # [16, D]
    WT = W.T  # [D, 16]
    w_hi = WT.astype(np.float16)
    w_lo = (WT - w_hi.astype(np.float32)).astype(np.float16)
    # [128, DC, 32] so the device DMA is contiguous 512B runs per partition
    whl = np.empty((128, DC, 32), dtype=np.float16)
    for c in range(DC):
        whl[:, c, 0:16] = w_hi[128 * c : 128 * (c + 1), :]
        whl[:, c, 16:32] = w_lo[128 * c : 128 * (c + 1), :]
    bcat = np.concatenate([gate_b, expert_b]).astype(np.float32)  # [16]
    brow = np.tile(bcat, 32)  # [512]
    fold = np.concatenate([np.eye(16), np.eye(16)], axis=0).astype(np.float16)
    ident = np.eye(128, dtype=np.float32)
    return {"whl": whl, "brow": brow, "fold": fold, "ident": ident}


_NC_CACHE = {}


def kernel(x, gate_w, gate_b, expert_w, expert_b, k):
    assert int(k) == 2
    x = np.ascontiguousarray(np.asarray(x, dtype=np.float32))
    assert x.shape == (B, D)

    from concourse.bass_utils import run_bass_kernel_spmd

    if B_LOC not in _NC_CACHE:
        _NC_CACHE[B_LOC] = build_module(B_LOC)
    nc = _NC_CACHE[B_LOC]

    common = host_inputs(
        np.asarray(gate_w, np.float32),
        np.asarray(gate_b, np.float32),
        np.asarray(expert_w, np.float32),
        np.asarray(expert_b, np.float32),
    )
    in_maps = [
        {**common, "x": x[i * B_LOC : (i + 1) * B_LOC]} for i in range(N_CORES)
    ]
    import os

    trace = bool(os.environ.get("MOE_TRACE"))
    if trace:
        _ensure_ntff_hook()
    res = run_bass_kernel_spmd(
        nc, in_maps, core_ids=list(range(N_CORES)), trace=trace
    )
    global LAST_RESULT
    LAST_RESULT = res
    out = np.concatenate([r["y"] for r in res.results])
    return out.reshape(B, 1).astype(np.float32)


LAST_RESULT = None


def _ensure_ntff_hook():
    """Register the axon NTFF profile hook if the antenv shim is missing
    (lets run_bass_kernel_spmd(trace=True) capture HW timing)."""
    try:
        import antenv.axon_hooks  # noqa: F401

        return
    except ImportError:
        pass
    try:
        import types

        import antenv
        from trn_agent_boot.trn_boot import _ntff_profile_via_ctypes

        mod = types.ModuleType("antenv.axon_hooks")
        _h = [None]
        mod.set_axon_ntff_profile_hook = lambda h: _h.__setitem__(0, h)
        mod.get_axon_ntff_profile_hook = lambda: _h[0]
        sys.modules["antenv.axon_hooks"] = mod
        antenv.axon_hooks = mod
        mod.set_axon_ntff_profile_hook(
            _ntff_profile_via_ctypes("/opt/axon/libaxon_pjrt.so")
        )
    except Exception as e:  # profiling is best-effort
        print(f"ntff hook setup failed: {e}")


if __name__ == "__main__":
    rng = np.random.default_rng(0)
    s = 1.0 / np.sqrt(D)
    inputs = {
        "x": rng.standard_normal((B, D), dtype=np.float32),
        "gate_w": rng.uniform(-s, s, (E, D)).astype(np.float32),
        "gate_b": rng.uniform(-s, s, E).astype(np.float32),
        "expert_w": rng.uniform(-s, s, (E, D)).astype(np.float32),
        "expert_b": rng.uniform(-s, s, E).astype(np.float32),
        "k": 2,
    }
    got = kernel(**inputs)
    print("kernel output:", got.shape, got.dtype, got[:4, 0])
